# revision 3
# baseline (speedup 1.0000x reference)
"""FreeYOLOv2 NMS-detection kernel for 8 Trainium2 NeuronCores.

Device (8 cores, SPMD): per-level, per-core candidate extraction over the
26M class logits (the heavy data-parallel scan): blocked DMA (1.3MB chunks,
overlapped with compute) -> grouped 16->1 reduce_max (single DVE pass) ->
max8 + max_index over the 16x-reduced block-max array (top-8 blocks per
partition-row). Exact: the global per-level top-1000 has <= 6 member blocks
per (core, partition-row) here (top-8 kept), and the host verifies a strict
certificate (8th block-max < 1000th value) with an exact fallback.

Host: merges the <=16k surviving candidates per level (exact top-1000 with
reference tie semantics), decodes DFL boxes for the 3000 winners, and runs
the tiny sequential greedy NMS, mirroring the reference formulas in fp32.
"""
import numpy as np

NUM_CLASSES = 80
REG_MAX = 16
TOPK = 1000
CONF_THRESH = 0.05
NMS_THRESH = 0.6
CLS_OFFSET = 8192.0

# (H, W, stride); per-core shard = H/8 rows
LEVELS = [(512, 512, 8.0), (256, 256, 16.0), (128, 128, 32.0)]
NCORES = 8

_COMPILED = None


def _build_kernel():
    from concourse import mybir, tile, bacc

    F32, U32 = mybir.dt.float32, mybir.dt.uint32
    nc = bacc.Bacc("TRN2", target_bir_lowering=False, debug=False,
                   num_devices=NCORES)

    G = 16          # reduce group: block-max granularity
    BW = 2560       # DMA/reduce block width (columns)
    ins = []
    outs = []
    specs = []  # (lname, ncol, nblk)
    for li, (H, W, _s) in enumerate(LEVELS):
        Hc = H // NCORES
        ncol = NUM_CLASSES * Hc * W // 128
        lname = f"l{li}"
        ins.append(nc.dram_tensor(f"cls_{lname}", [128, ncol], F32,
                                  kind="ExternalInput"))
        outs.append((
            nc.dram_tensor(f"val_{lname}", [128, 8], F32,
                           kind="ExternalOutput"),
            nc.dram_tensor(f"idx_{lname}", [128, 8], U32,
                           kind="ExternalOutput"),
        ))
        specs.append((lname, ncol, G))

    with tile.TileContext(nc) as tc:
        with tc.tile_pool(name="p", bufs=1) as pool:
            # emit P3 first so its (larger) extraction overlaps later DMAs
            red_tiles = {}
            for li, (lname, ncol, _g) in enumerate(specs):
                nred = ncol // G
                red = pool.tile([128, nred], F32, tag=f"r{li}")
                red_tiles[li] = red
                nblk = (ncol + BW - 1) // BW
                for b in range(nblk):
                    base = b * BW
                    size = min(BW, ncol - base)
                    x = pool.tile([128, size], F32, tag=f"x{li}_{b}")
                    nc.sync.dma_start(out=x[:, :], in_=ins[li][:, base:base + size])
                    nc.vector.tensor_reduce(
                        out=red[:, base // G:(base + size) // G],
                        in_=x.rearrange("p (a b) -> p a b", b=G),
                        axis=mybir.AxisListType.X, op=mybir.AluOpType.max)
            for li, (lname, ncol, _g) in enumerate(specs):
                red = red_tiles[li]
                v = pool.tile([128, 8], F32, tag=f"v{li}")
                ix = pool.tile([128, 8], U32, tag=f"i{li}")
                nc.vector.max(out=v[:, :], in_=red[:, :])
                nc.vector.max_index(out=ix[:, :], in_max=v[:, :],
                                    in_values=red[:, :])
                nc.sync.dma_start(out=outs[li][0][:, :], in_=v[:, :])
                nc.sync.dma_start(out=outs[li][1][:, :], in_=ix[:, :])
    nc.compile()
    return nc, specs


def _get_compiled():
    global _COMPILED
    if _COMPILED is None:
        _COMPILED = _build_kernel()
    return _COMPILED


def _run_device(inputs):
    """Shard inputs, run the SPMD extraction kernel on 8 cores, return
    per-core candidate (value, shard-flat-index) arrays per level."""
    from concourse.bass_utils import run_bass_kernel_spmd

    nc, specs = _get_compiled()
    in_maps = []
    for c in range(NCORES):
        m = {}
        for li, (H, W, _s) in enumerate(LEVELS):
            Hc = H // NCORES
            cls = inputs[f'cls_p{li + 3}']  # [1, C, H, W]
            shard = np.ascontiguousarray(cls[0, :, c * Hc:(c + 1) * Hc, :])
            ncol = NUM_CLASSES * Hc * W // 128
            m[f"cls_l{li}"] = shard.reshape(128, ncol)
        in_maps.append(m)
    res = run_bass_kernel_spmd(nc, in_maps, list(range(NCORES)))
    return res.results, specs


def _sigmoid(x):
    return np.where(x >= 0, 1.0 / (1.0 + np.exp(-x)),
                    np.exp(x) / (1.0 + np.exp(x))).astype(np.float32)


def kernel(cls_p3, reg_p3, cls_p4, reg_p4, cls_p5, reg_p5, proj_w):
    inputs = {'cls_p3': np.asarray(cls_p3), 'reg_p3': np.asarray(reg_p3),
              'cls_p4': np.asarray(cls_p4), 'reg_p4': np.asarray(reg_p4),
              'cls_p5': np.asarray(cls_p5), 'reg_p5': np.asarray(reg_p5),
              'proj_w': np.asarray(proj_w)}
    results, specs = _run_device(inputs)

    all_s, all_l, all_b = [], [], []
    G = 16
    for li, (H, W, stride) in enumerate(LEVELS):
        Hc = H // NCORES
        ncol = NUM_CLASSES * Hc * W // 128
        vals_l, flats_l = [], []
        for c in range(NCORES):
            blk = results[c][f"idx_l{li}"].astype(np.int64)   # [128, 8] block ids
            cls = inputs[f'cls_p{li + 3}']
            shard = np.ascontiguousarray(cls[0, :, c * Hc:(c + 1) * Hc, :]).reshape(128, ncol)
            # expand each selected block to its G raw elements
            col = (blk[:, :, None] * G + np.arange(G)[None, None, :]).reshape(128, 8 * G)
            v = np.take_along_axis(shard, col, axis=1)
            shard_flat = np.arange(128)[:, None] * ncol + col
            cidx = shard_flat // (Hc * W)
            rem = shard_flat % (Hc * W)
            h = c * Hc + rem // W
            w = rem % W
            flat_ref = (h * W + w) * NUM_CLASSES + cidx
            vals_l.append(v.ravel())
            flats_l.append(flat_ref.ravel())
        vals = np.concatenate(vals_l)
        flats = np.concatenate(flats_l)
        # exact reference top-1000: stable argsort of -sigmoid == (-logit, flat)
        order = np.lexsort((flats, -vals))[:TOPK]
        top_logit = vals[order]
        top_flat = flats[order]
        # Exactness certificate: a top-1000 member can only be missed if some
        # row held >8 blocks with max >= theta (the 1000th value). Then that
        # row's 8th extracted block-max would be >= theta. Verify none is.
        theta = top_logit[-1]
        eighth = np.stack([results[c][f"val_l{li}"][:, 7] for c in range(NCORES)])
        if (eighth >= theta).any():
            # fall back to an exact host recompute for this level (never taken
            # for the verified input family; guards arbitrary inputs)
            cls = inputs[f'cls_p{li + 3}']
            flat_all = cls[0].transpose(1, 2, 0).reshape(-1)
            order_f = np.lexsort((np.arange(flat_all.size), -flat_all))[:TOPK]
            top_logit = flat_all[order_f]
            top_flat = order_f
        scores = _sigmoid(top_logit)
        anchor = top_flat // NUM_CLASSES
        labels = (top_flat % NUM_CLASSES).astype(np.int32)
        # DFL decode for the selected anchors only (fp32, reference formula)
        reg = inputs[f'reg_p{li + 3}'][0]              # [64, H, W]
        hh, ww = anchor // W, anchor % W
        r = reg[:, hh, ww].T.reshape(-1, 4, REG_MAX).astype(np.float32)
        m = r.max(axis=-1, keepdims=True)
        e = np.exp(r - m)
        sm = (e / e.sum(axis=-1, keepdims=True)).astype(np.float32)
        dist = (sm * np.asarray(proj_w, np.float32)).sum(-1).astype(np.float32)
        ax = (ww.astype(np.float32) + 0.5) * np.float32(stride)
        ay = (hh.astype(np.float32) + 0.5) * np.float32(stride)
        boxes = np.stack([ax - dist[:, 0] * np.float32(stride),
                          ay - dist[:, 1] * np.float32(stride),
                          ax + dist[:, 2] * np.float32(stride),
                          ay + dist[:, 3] * np.float32(stride)], -1)
        sc = np.where(scores > CONF_THRESH, scores, 0.0).astype(np.float32)
        all_s.append(sc)
        all_l.append(labels)
        all_b.append(boxes.astype(np.float32))

    scores = np.concatenate(all_s)
    labels = np.concatenate(all_l)
    boxes = np.concatenate(all_b)

    # global stable sort by -score (ties by concat position == reference)
    order = np.argsort(-scores, kind='stable')
    scores, labels, boxes = scores[order], labels[order], boxes[order]

    shifted = boxes + (labels.astype(np.float32) * np.float32(CLS_OFFSET))[:, None]
    x1, y1, x2, y2 = shifted[:, 0], shifted[:, 1], shifted[:, 2], shifted[:, 3]
    area = (x2 - x1) * (y2 - y1)
    N = scores.shape[0]
    keep = scores > 0.0
    idx = np.arange(N)
    for i in range(N):
        if not keep[i]:
            continue
        iw = np.minimum(x2[i], x2) - np.maximum(x1[i], x1)
        ih = np.minimum(y2[i], y2) - np.maximum(y1[i], y1)
        iw = np.clip(iw, 0.0, None).astype(np.float32)
        ih = np.clip(ih, 0.0, None).astype(np.float32)
        inter = iw * ih
        iou = inter / (area[i] + area - inter + np.float32(1e-9))
        sup = (iou > NMS_THRESH) & keep & (idx > i)
        keep = keep & ~sup
    return (boxes.astype(np.float32),
            (scores * keep).astype(np.float32),
            labels.astype(np.int32),
            keep)
